# revision 10
# baseline (speedup 1.0000x reference)
"""GCN link-prediction kernel for 8 Trainium2 NeuronCores (Bass/Tile).

Strategy (target-sharded message passing):
- Nodes are sharded 8 ways.  Each core computes z1 = x @ W1 for its
  shard, scales by dinv, and AllGathers a bf16 message table zt1.
- Edges are sharded by target core.  Each core aggregates its in-edges
  with bulk int16 dma_gather calls (4 SWDGE queues, source chunked 4x to
  fit int16 indexing), then a selection-matrix matmul segment-sum into
  per-window PSUM tiles accumulating into an SBUF aggregate.  Layer 2
  repeats the same structure on the layer-1 output.
- Edge scoring gathers final embeddings h2 for src/dst (bucketed by
  (src_chunk, dst_chunk) so both gathers share slot order), computes
  relu((h_src - h_dst) @ w + b) per edge, and AllReduces the loss mean.
- All graph-structure preprocessing (index building, padding plans) is
  host-side integer work; all floating-point math runs on device.
"""
import json
import os
import sys
import types

sys.path.insert(0, "/opt/trn_rl_repo")

import numpy as np
import ml_dtypes

import concourse.mybir as mybir
import concourse.tile as tile
from concourse import bacc
from concourse.bass_utils import run_bass_kernel_spmd

F32 = mybir.dt.float32
BF16 = mybir.dt.bfloat16
I16 = mybir.dt.int16

NC = 8            # cores
N = 100000        # nodes
F_IN = 512        # input features
H = 128           # hidden width
C = 64            # output width
NSH = 12500       # real nodes per core
W = 98            # 128-node target windows per core
NSP = W * 128     # padded nodes per core
TR = NSP * NC     # message-table rows
K = 4             # source chunks (each < 32768 rows for int16 gather)
CH = TR // K
GCALL = 4096      # max slots per dma_gather call
GPG = GCALL // 128
PADU = 1024       # call-size quantum
NQ = 4            # SWDGE queues

LAST_RESULTS = None  # test harness reads exec_time_ns from here


def _set_cfg(n, nsh, w, k, gcall, padu):
    """Override problem size (for scaled-down simulator tests)."""
    global N, NSH, W, NSP, TR, K, CH, GCALL, GPG, PADU
    N, NSH, W, K, GCALL, PADU = n, nsh, w, k, gcall, padu
    NSP = W * 128
    TR = NSP * NC
    CH = TR // K
    GPG = GCALL // 128


def _install_ntff_hook():
    try:
        import antenv
        if getattr(antenv, "axon_hooks", None) is not None:
            return
        mod = types.ModuleType("antenv.axon_hooks")
        _h = [None]
        mod.set_axon_ntff_profile_hook = lambda h: _h.__setitem__(0, h)
        mod.get_axon_ntff_profile_hook = lambda: _h[0]
        sys.modules["antenv.axon_hooks"] = mod
        antenv.axon_hooks = mod
        if "/root/.axon_site" not in sys.path:
            sys.path.insert(0, "/root/.axon_site")
        from trn_agent_boot.trn_boot import _ntff_profile_via_ctypes
        mod.set_axon_ntff_profile_hook(
            _ntff_profile_via_ctypes("/opt/axon/libaxon_pjrt.so"))
    except Exception:
        pass


# ------------------------------------------------- BIR wait-split fixup

_CAP = {"EventSemaphore": 2}
_EV_MAX = 2


def fix_dma_waits(nc) -> int:
    """Post-finalize fixups.

    1) SWDGE queue_num must match the DMASW sem lane Tile assigned
       (each lane is hardware-locked to one queue) -- mutate in place.
    2) walrus allows ~1 sync wait per instruction; if any non-EventSemaphore
       instruction still carries more (Bacc normally splits these), hoist
       extras onto EventSemaphores via a JSON round-trip.
    """
    n_queue = 0
    n_multi = 0
    for f in nc.m.functions:
        for bb in f.blocks:
            for ins in bb.instructions:
                op = type(ins).__name__
                si = ins.sync_info
                if op in ("InstDMAGatherAnt", "InstDMAScatterAddAnt") and si:
                    for u in si.on_update:
                        nm = u.ant_name or ""
                        if nm.startswith("DMASW"):
                            lane = int(nm[5:nm.index("_")])
                            if ins.queue_num != lane % NQ:
                                ins.queue_num = lane % NQ
                                n_queue += 1
                if si and len(si.on_wait) > (2 if op == "InstEventSemaphore" else 1):
                    n_multi += 1
    if n_multi:
        # fallback: JSON-level wait splitting (loses sim-only state; the
        # HW path re-parses from JSON anyway)
        js = json.loads(mybir.module_to_json_string(nc.m))
        for f in js["functions"]:
            for bb in f["blocks"]:
                out = []
                for ins in bb["instructions"]:
                    si = ins.get("sync_info") or {}
                    waits = si.get("on_wait") or []
                    cap = _CAP.get(ins.get("opcode"), 1)
                    if len(waits) > cap:
                        extra = waits[:-cap]
                        si["on_wait"] = waits[-cap:]
                        kk = 0
                        while extra:
                            chunk, extra = extra[:_EV_MAX], extra[_EV_MAX:]
                            out.append({
                                "debug": ins.get("debug"),
                                "engine": ins["engine"],
                                "ins": [], "outs": [],
                                "name": f"{ins['name']}_wsplit{kk}",
                                "opcode": "EventSemaphore",
                                "sync_info": {"on_update": [], "on_wait": chunk},
                            })
                            kk += 1
                    out.append(ins)
                bb["instructions"] = out
        nc.m = mybir.module_from_json_string(json.dumps(js))
    return n_queue + n_multi


# ------------------------------------------------------- host planning

def _wrap_call(flat):
    n = len(flat)
    return flat.reshape(n // 16, 16).T.astype(np.int16)


def _pad_to(x, unit):
    return -(-x // unit) * unit


class Plan:
    pass


def _build_plan(x, edge_index, pos_edge_index, neg_edge_index):
    p = Plan()
    deg = np.bincount(edge_index[1], minlength=N).astype(np.int64) + 1

    # node -> (core, window, slot); degree-balanced windows
    rowmap = np.empty(N, np.int64)
    degarr = np.ones((NC, 128, W), np.float32)
    for c in range(NC):
        nodes = np.arange(c * NSH, (c + 1) * NSH)
        order = np.argsort(-deg[nodes], kind="stable")
        wv = np.arange(NSH) % W
        tv = np.arange(NSH) // W
        loc = np.empty(NSH, np.int64)
        loc[order] = wv * 128 + tv
        rowmap[nodes] = c * NSP + loc
        degarr[c, tv, wv] = deg[nodes[order]].astype(np.float32)
    p.rowmap = rowmap
    p.degarr = degarr

    # one dummy (padding) table row per source chunk: all dummies have
    # identical embeddings, so scoring pads (src-dummy minus dst-dummy)
    # contribute exactly zero
    used = np.zeros(TR, bool)
    used[rowmap] = True
    dummy_local = np.empty(K, np.int64)
    for k in range(K):
        free = np.flatnonzero(~used[k * CH:(k + 1) * CH])
        assert len(free) > 0, "no dummy rows in chunk"
        dummy_local[k] = free[0]
    p.dummy_local = dummy_local

    # ---- aggregation slots ----
    row = edge_index[0].astype(np.int64)
    col = edge_index[1].astype(np.int64)
    ecore = col // NSH
    srow = rowmap[row]
    tloc = rowmap[col] - ecore * NSP
    ew, et = tloc // 128, tloc % 128
    ek, eli = srow // CH, srow % CH

    gid = (ecore * K + ek) * W + ew
    cnt = np.bincount(gid, minlength=NC * K * W).reshape(NC, K, W)
    pages_kw = (-(-cnt // 128)).max(axis=0)      # [K, W]

    woff = np.zeros((K, W), np.int64)
    chunk_ranges = []
    off = 0
    for k in range(K):
        base = off
        for w in range(W):
            woff[k, w] = off
            off += 128 * int(pages_kw[k, w])
        off = base + _pad_to(off - base, PADU)
        chunk_ranges.append((base, off))
    TOT = off
    p.agg_tot = TOT

    gidx = np.zeros((NC, TOT), np.int64)
    gtgt = np.full((NC, TOT), -1.0, np.float32)
    eorder = np.lexsort((et, ew, ek, ecore))
    sc, sk, sw_ = ecore[eorder], ek[eorder], ew[eorder]
    sli, st_ = eli[eorder], et[eorder]
    g_srt = (sc * K + sk) * W + sw_
    grp_start = np.zeros(NC * K * W, np.int64)
    grp_start[1:] = np.cumsum(np.bincount(g_srt, minlength=NC * K * W))[:-1]
    pos_in_grp = np.arange(len(eorder)) - grp_start[g_srt]
    slot = woff[sk, sw_] + pos_in_grp
    gidx[sc, slot] = sli
    gtgt[sc, slot] = st_.astype(np.float32)

    agg_calls = []
    for k in range(K):
        base, end = chunk_ranges[k]
        pw = np.full((end - base) // 128, -1, np.int64)
        for w in range(W):
            s0 = (woff[k, w] - base) // 128
            pw[s0:s0 + int(pages_kw[k, w])] = w
        s = base
        while s < end:
            nidx = min(GCALL, end - s)
            npg = nidx // 128
            pg0 = (s - base) // 128
            pages = []
            for j in range(npg):
                w = int(pw[pg0 + j])
                if w < 0:
                    pages.append((-1, True, True))
                else:
                    w0 = (woff[k, w] - base) // 128
                    pages.append((w, pg0 + j == w0,
                                  pg0 + j == w0 + int(pages_kw[k, w]) - 1))
            agg_calls.append((k, nidx, s // 16, s // 128, pages))
            s += nidx
    p.agg_calls = agg_calls

    gidx_w = np.zeros((NC, 16, TOT // 16), np.int16)
    for (k, nidx, coff, poff, pages) in agg_calls:
        s = coff * 16
        for c in range(NC):
            gidx_w[c][:, coff:coff + nidx // 16] = _wrap_call(gidx[c, s:s + nidx])
    p.gidx_in = np.ascontiguousarray(np.tile(gidx_w, (1, 8, 1)))
    p.gtgt_in = np.ascontiguousarray(
        gtgt.reshape(NC, TOT // 128, 128).transpose(0, 2, 1))

    # ---- scoring ----
    tot_ei = np.concatenate([pos_edge_index, neg_edge_index], axis=1).astype(np.int64)
    EP = tot_ei.shape[1]
    p.ep = EP
    EPC = EP // NC
    sr = rowmap[tot_ei[0]]
    dr = rowmap[tot_ei[1]]
    score_core = np.arange(EP) // EPC
    sbk = (sr // CH) * K + (dr // CH)
    scnt = np.zeros((NC, K * K), np.int64)
    np.add.at(scnt, (score_core, sbk), 1)
    bslots = np.array([_pad_to(int(v), PADU) for v in scnt.max(axis=0)], np.int64)
    boff = np.zeros(K * K + 1, np.int64)
    boff[1:] = np.cumsum(bslots)
    SSLOT = int(boff[-1])
    p.sslot = SSLOT

    sidx_s = np.zeros((NC, SSLOT), np.int64)
    sidx_d = np.zeros((NC, SSLOT), np.int64)
    for b in range(K * K):
        sidx_s[:, boff[b]:boff[b + 1]] = dummy_local[b // K]
        sidx_d[:, boff[b]:boff[b + 1]] = dummy_local[b % K]
    slot2edge = np.full((NC, SSLOT), -1, np.int64)
    sord = np.lexsort((np.arange(EP), sbk, score_core))
    sc_s, sb_s = score_core[sord], sbk[sord]
    gb = sc_s * (K * K) + sb_s
    gb_start = np.zeros(NC * K * K, np.int64)
    gb_start[1:] = np.cumsum(np.bincount(gb, minlength=NC * K * K))[:-1]
    pos_b = np.arange(EP) - gb_start[gb]
    sslot_ = boff[sb_s] + pos_b
    sidx_s[sc_s, sslot_] = sr[sord] % CH
    sidx_d[sc_s, sslot_] = dr[sord] % CH
    slot2edge[sc_s, sslot_] = sord
    p.slot2edge = slot2edge

    score_calls = []
    for b in range(K * K):
        s, end = int(boff[b]), int(boff[b + 1])
        while s < end:
            nidx = min(GCALL, end - s)
            score_calls.append((b // K, b % K, nidx, s // 16, s))
            s += nidx
    p.score_calls = score_calls

    sidx_s_w = np.zeros((NC, 16, SSLOT // 16), np.int16)
    sidx_d_w = np.zeros((NC, 16, SSLOT // 16), np.int16)
    for (ks, kd, nidx, coff, soff) in score_calls:
        for c in range(NC):
            sidx_s_w[c][:, coff:coff + nidx // 16] = _wrap_call(sidx_s[c, soff:soff + nidx])
            sidx_d_w[c][:, coff:coff + nidx // 16] = _wrap_call(sidx_d[c, soff:soff + nidx])
    p.sidx_s_in = np.ascontiguousarray(np.tile(sidx_s_w, (1, 8, 1)))
    p.sidx_d_in = np.ascontiguousarray(np.tile(sidx_d_w, (1, 8, 1)))

    # ---- xT per core ----
    xT = np.zeros((NC, F_IN, NSP), ml_dtypes.bfloat16)
    for c in range(NC):
        nodes = np.arange(c * NSH, (c + 1) * NSH)
        lp = rowmap[nodes] - c * NSP
        xt = np.zeros((NSP, F_IN), np.float32)
        xt[lp] = x[nodes]
        xT[c] = np.ascontiguousarray(xt.T).astype(ml_dtypes.bfloat16)
    p.xT = xT
    return p


# ------------------------------------------------------- device program

def _emit_agg(nc, p, iop, gp, sp, pp, gidx, gtgt, table, regs, iota_t,
              agg, nout, psum_tag):
    """Shared gather + selection-matmul aggregation loop (layers 1 & 2)."""
    psum = None
    for ci, (k, nidx, coff, poff, pages) in enumerate(p.agg_calls):
        npg = nidx // 128
        it = iop.tile([128, GCALL // 16], I16, tag="i0")
        nc.sync.dma_start(it[:, :nidx // 16], gidx[:, coff:coff + nidx // 16])
        tg = iop.tile([128, GPG], F32, tag="t0")
        nc.sync.dma_start(tg[:, :npg], gtgt[:, poff:poff + npg])
        gt = gp.tile([128, GPG, H], BF16, tag="g0")
        nc.gpsimd.dma_gather(
            out_ap=gt[:, :npg, :], in_ap=table[k * CH:(k + 1) * CH, :],
            idxs_ap=it[:, :nidx // 16], num_idxs=nidx, num_idxs_reg=regs[nidx],
            elem_size=H, single_packet=False, queue_num=ci % NQ)
        st = sp.tile([128, GPG, 128], BF16, tag="s0")
        nc.vector.tensor_tensor(
            out=st[:, :npg, :],
            in0=tg[:, :npg].unsqueeze(2).broadcast_to([128, npg, 128]),
            in1=iota_t[:].unsqueeze(1).broadcast_to([128, npg, 128]),
            op=mybir.AluOpType.is_equal)
        for j, (w, first, last) in enumerate(pages):
            if w < 0:
                continue
            if first:
                psum = pp.tile([128, nout], F32, tag=psum_tag)
            nc.tensor.matmul(psum[:], lhsT=st[:, j, :], rhs=gt[:, j, :nout],
                             start=first, stop=last)
            if last:
                nc.vector.tensor_tensor(
                    out=agg[:, w * nout:(w + 1) * nout],
                    in0=agg[:, w * nout:(w + 1) * nout],
                    in1=psum[:], op=mybir.AluOpType.add)


def _build_nc(p):
    nc = bacc.Bacc(None, target_bir_lowering=False, debug=False,
                   num_devices=NC, num_swdge_queues=NQ)
    TOT = p.agg_tot
    SSLOT = p.sslot

    xT = nc.declare_dram_parameter("xT", [F_IN, NSP], BF16, isOutput=False)
    w1 = nc.declare_dram_parameter("w1", [F_IN, H], BF16, isOutput=False)
    w2 = nc.declare_dram_parameter("w2", [H, C], BF16, isOutput=False)
    b1r = nc.declare_dram_parameter("b1r", [128, H], F32, isOutput=False)
    b2r = nc.declare_dram_parameter("b2r", [128, C], F32, isOutput=False)
    swr = nc.declare_dram_parameter("swr", [128, C], F32, isOutput=False)
    sbr = nc.declare_dram_parameter("sbr", [128, 1], F32, isOutput=False)
    iota = nc.declare_dram_parameter("iota", [128, 128], F32, isOutput=False)
    ident = nc.declare_dram_parameter("ident", [128, 128], F32, isOutput=False)
    onesc = nc.declare_dram_parameter("onesc", [128, 1], F32, isOutput=False)
    degp = nc.declare_dram_parameter("degp", [128, W], F32, isOutput=False)
    gidx = nc.declare_dram_parameter("gidx", [128, TOT // 16], I16, isOutput=False)
    gtgt = nc.declare_dram_parameter("gtgt", [128, TOT // 128], F32, isOutput=False)
    sidxs = nc.declare_dram_parameter("sidxs", [128, SSLOT // 16], I16, isOutput=False)
    sidxd = nc.declare_dram_parameter("sidxd", [128, SSLOT // 16], I16, isOutput=False)

    out_s = nc.declare_dram_parameter("out_s", [SSLOT], F32, isOutput=True)
    out_loss = nc.declare_dram_parameter("out_loss", [1, 1], F32, isOutput=True)

    rg = [list(range(NC))]

    with tile.TileContext(nc) as tc:
        with (
            tc.tile_pool(name="const", bufs=1) as cp,
            tc.tile_pool(name="dram", bufs=1, space="DRAM") as dp,
            tc.tile_pool(name="io", bufs=4) as iop,
            tc.tile_pool(name="gath", bufs=3) as gp,
            tc.tile_pool(name="sel", bufs=2) as sp,
            tc.tile_pool(name="fin", bufs=4) as fp,
        ):
            regs = {}
            for (_, nidx, _, _, _) in p.agg_calls:
                regs.setdefault(nidx, None)
            for (_, _, nidx, _, _) in p.score_calls:
                regs.setdefault(nidx, None)
            for nidx in regs:
                regs[nidx] = nc.gpsimd.to_reg(nidx)

            w1_t = cp.tile([128, F_IN // 128, H], BF16)
            nc.sync.dma_start(w1_t[:], w1[:].rearrange("(k p) h -> p k h", p=128))
            w2_t = cp.tile([128, C], BF16)
            nc.sync.dma_start(w2_t[:], w2[:])
            b1_t = cp.tile([128, H], F32)
            nc.sync.dma_start(b1_t[:], b1r[:])
            b2_t = cp.tile([128, C], F32)
            nc.sync.dma_start(b2_t[:], b2r[:])
            sw_t = cp.tile([128, C], F32)
            nc.sync.dma_start(sw_t[:], swr[:])
            sb_t = cp.tile([128, 1], F32)
            nc.sync.dma_start(sb_t[:], sbr[:])
            iota_t = cp.tile([128, 128], F32)
            nc.sync.dma_start(iota_t[:], iota[:])
            id_t = cp.tile([128, 128], F32)
            nc.sync.dma_start(id_t[:], ident[:])
            ones_t = cp.tile([128, 1], F32)
            nc.sync.dma_start(ones_t[:], onesc[:])
            deg_t = cp.tile([128, W], F32)
            nc.sync.dma_start(deg_t[:], degp[:])
            dinv_t = cp.tile([128, W], F32)
            dsq_t = cp.tile([128, W], F32)
            nc.scalar.activation(dsq_t[:], deg_t[:],
                                 mybir.ActivationFunctionType.Sqrt)
            nc.vector.reciprocal(dinv_t[:], dsq_t[:])

            zt1loc = dp.tile([NSP, H], BF16, tag="zt1loc")
            zt1tab = dp.tile([TR, H], BF16, tag="zt1tab")
            zt2loc = dp.tile([NSP, 128], BF16, tag="zt2loc")
            zt2tab = dp.tile([TR, 128], BF16, tag="zt2tab")
            h2loc = dp.tile([NSP, C], F32, tag="h2loc")
            h2tab = dp.tile([TR, C], F32, tag="h2tab")
            lossloc = dp.tile([1, 1], F32, tag="lossloc")
            lossout = dp.tile([1, 1], F32, tag="lossout")

            with tc.tile_pool(name="pAB", bufs=1) as pab:
                zt1own = pab.tile([128, NSP], BF16, tag="zt1own")
                hT = pab.tile([128, NSP], BF16, tag="hT")

                # ---- phase A ----
                with nc.named_scope("phA"), \
                        tc.tile_pool(name="pA", bufs=1) as pa, \
                        tc.tile_pool(name="psA", bufs=4, space="PSUM") as ppa:
                    xts = []
                    for kk in range(F_IN // 128):
                        xt_t = pa.tile([128, NSP], BF16, tag=f"xt{kk}")
                        nc.sync.dma_start(xt_t[:], xT[kk * 128:(kk + 1) * 128, :])
                        xts.append(xt_t)
                    for w in range(W):
                        ps1 = ppa.tile([128, H], F32, tag="psA")
                        for kk in range(F_IN // 128):
                            nc.tensor.matmul(
                                ps1[:], lhsT=xts[kk][:, w * 128:(w + 1) * 128],
                                rhs=w1_t[:, kk, :], start=(kk == 0),
                                stop=(kk == F_IN // 128 - 1))
                        nc.vector.tensor_tensor(
                            out=zt1own[:, w * 128:(w + 1) * 128], in0=ps1[:],
                            in1=dinv_t[:, w:w + 1].broadcast_to([128, H]),
                            op=mybir.AluOpType.mult)
                    nc.sync.dma_start(
                        zt1loc[:].rearrange("(w t) h -> t w h", t=128),
                        zt1own[:])
                    nc.gpsimd.collective_compute(
                        "AllGather", mybir.AluOpType.bypass, replica_groups=rg,
                        ins=[zt1loc.opt()], outs=[zt1tab.opt()])

                # ---- phase B ----
                with nc.named_scope("phB"), \
                        tc.tile_pool(name="pB", bufs=1) as pb, \
                        tc.tile_pool(name="psB", bufs=3, space="PSUM") as ppb:
                    agg1 = pb.tile([128, NSP], F32, tag="agg1")
                    nc.vector.memset(agg1[:], 0.0)
                    _emit_agg(nc, p, iop, gp, sp, ppb, gidx, gtgt, zt1tab,
                              regs, iota_t, agg1, H, "mm1")
                    for w in range(W):
                        sl = slice(w * 128, (w + 1) * 128)
                        t0 = fp.tile([128, H], F32, tag="f0")
                        nc.vector.tensor_tensor(
                            out=t0[:], in0=agg1[:, sl], in1=zt1own[:, sl],
                            op=mybir.AluOpType.add)
                        nc.vector.tensor_tensor(
                            out=t0[:], in0=t0[:],
                            in1=dinv_t[:, w:w + 1].broadcast_to([128, H]),
                            op=mybir.AluOpType.mult)
                        nc.vector.tensor_tensor(
                            out=t0[:], in0=t0[:], in1=b1_t[:],
                            op=mybir.AluOpType.add)
                        nc.vector.tensor_scalar_max(t0[:], t0[:], 0.0)
                        pst = ppb.tile([128, 128], F32, tag="psT")
                        nc.tensor.transpose(pst[:], t0[:], id_t[:])
                        nc.vector.tensor_copy(hT[:, sl], pst[:])

                # ---- phase C part 1: z2, zt2 table ----
                pc_stack = tc.tile_pool(name="pC", bufs=1)
                pcp = pc_stack.__enter__()
                with nc.named_scope("phC1"), \
                        tc.tile_pool(name="psC", bufs=4, space="PSUM") as ppc:
                    zt2pad = pcp.tile([128, NSP], BF16, tag="zt2pad")
                    nc.vector.memset(zt2pad[:], 0.0)
                    for w in range(W):
                        ps2 = ppc.tile([128, C], F32, tag="psC")
                        nc.tensor.matmul(ps2[:], lhsT=hT[:, w * 128:(w + 1) * 128],
                                         rhs=w2_t[:], start=True, stop=True)
                        nc.vector.tensor_tensor(
                            out=zt2pad[:, w * 128:w * 128 + C], in0=ps2[:],
                            in1=dinv_t[:, w:w + 1].broadcast_to([128, C]),
                            op=mybir.AluOpType.mult)
                    nc.sync.dma_start(
                        zt2loc[:].rearrange("(w t) h -> t w h", t=128),
                        zt2pad[:])
                    nc.gpsimd.collective_compute(
                        "AllGather", mybir.AluOpType.bypass, replica_groups=rg,
                        ins=[zt2loc.opt()], outs=[zt2tab.opt()])

                # ---- phase C part 2: layer-2 aggregation ----
                with nc.named_scope("phC2"), \
                        tc.tile_pool(name="psC2", bufs=3, space="PSUM") as ppc2:
                    agg2 = pcp.tile([128, W * C], F32, tag="agg2")
                    nc.vector.memset(agg2[:], 0.0)
                    _emit_agg(nc, p, iop, gp, sp, ppc2, gidx, gtgt, zt2tab,
                              regs, iota_t, agg2, C, "mm2")
                    h2own = pcp.tile([128, W * C], F32, tag="h2own")
                    for w in range(W):
                        slh = slice(w * C, (w + 1) * C)
                        t0 = fp.tile([128, C], F32, tag="f2")
                        nc.vector.tensor_tensor(
                            out=t0[:], in0=agg2[:, slh],
                            in1=zt2pad[:, w * 128:w * 128 + C],
                            op=mybir.AluOpType.add)
                        nc.vector.tensor_tensor(
                            out=t0[:], in0=t0[:],
                            in1=dinv_t[:, w:w + 1].broadcast_to([128, C]),
                            op=mybir.AluOpType.mult)
                        nc.vector.tensor_tensor(
                            out=h2own[:, slh], in0=t0[:], in1=b2_t[:],
                            op=mybir.AluOpType.add)
                    nc.sync.dma_start(
                        h2loc[:].rearrange("(w t) h -> t w h", t=128),
                        h2own[:])
                    nc.gpsimd.collective_compute(
                        "AllGather", mybir.AluOpType.bypass, replica_groups=rg,
                        ins=[h2loc.opt()], outs=[h2tab.opt()])
                pc_stack.__exit__(None, None, None)

            # ---- phase D: scoring ----
            with nc.named_scope("phD"), \
                    tc.tile_pool(name="pD", bufs=2) as pd, \
                    tc.tile_pool(name="psD", bufs=2, space="PSUM") as ppd:
                lacc = cp.tile([128, 1], F32)
                nc.vector.memset(lacc[:], 0.0)
                for ci, (ks, kd, nidx, coff, soff) in enumerate(p.score_calls):
                    npg = nidx // 128
                    its = iop.tile([128, GCALL // 16], I16, tag="i0")
                    nc.sync.dma_start(its[:, :nidx // 16],
                                      sidxs[:, coff:coff + nidx // 16])
                    itd = iop.tile([128, GCALL // 16], I16, tag="i1")
                    nc.sync.dma_start(itd[:, :nidx // 16],
                                      sidxd[:, coff:coff + nidx // 16])
                    gs = gp.tile([128, GPG, C], F32, tag="g0")
                    nc.gpsimd.dma_gather(
                        out_ap=gs[:, :npg, :], in_ap=h2tab[ks * CH:(ks + 1) * CH, :],
                        idxs_ap=its[:, :nidx // 16], num_idxs=nidx,
                        num_idxs_reg=regs[nidx], elem_size=C,
                        single_packet=False, queue_num=(2 * ci) % NQ)
                    gd = pd.tile([128, GPG, C], F32, tag="g1")
                    nc.gpsimd.dma_gather(
                        out_ap=gd[:, :npg, :], in_ap=h2tab[kd * CH:(kd + 1) * CH, :],
                        idxs_ap=itd[:, :nidx // 16], num_idxs=nidx,
                        num_idxs_reg=regs[nidx], elem_size=C,
                        single_packet=False, queue_num=(2 * ci + 1) % NQ)
                    d_t = pd.tile([128, GPG, C], F32, tag="d0")
                    nc.vector.tensor_tensor(
                        out=d_t[:, :npg, :], in0=gs[:, :npg, :],
                        in1=gd[:, :npg, :], op=mybir.AluOpType.subtract)
                    nc.vector.tensor_tensor(
                        out=d_t[:, :npg, :], in0=d_t[:, :npg, :],
                        in1=sw_t[:].unsqueeze(1).broadcast_to([128, npg, C]),
                        op=mybir.AluOpType.mult)
                    s_t = fp.tile([128, GPG], F32, tag="s0f")
                    nc.vector.reduce_sum(out=s_t[:, :npg], in_=d_t[:, :npg, :],
                                         axis=mybir.AxisListType.X)
                    ls = fp.tile([128, 1], F32, tag="ls")
                    nc.vector.reduce_sum(out=ls[:], in_=s_t[:, :npg],
                                         axis=mybir.AxisListType.X)
                    nc.vector.tensor_tensor(out=lacc[:], in0=lacc[:], in1=ls[:],
                                            op=mybir.AluOpType.add)
                    o_t = fp.tile([128, GPG], F32, tag="o0")
                    nc.vector.tensor_tensor(
                        out=o_t[:, :npg], in0=s_t[:, :npg],
                        in1=sb_t[:].broadcast_to([128, npg]),
                        op=mybir.AluOpType.add)
                    nc.vector.tensor_scalar_max(o_t[:, :npg], o_t[:, :npg], 0.0)
                    nc.sync.dma_start(
                        out_s[soff:soff + nidx].rearrange("(g t) -> t g", t=128),
                        o_t[:, :npg])
                psl = ppd.tile([1, 1], F32, tag="psL")
                nc.tensor.matmul(psl[:], lhsT=ones_t[:], rhs=lacc[:],
                                 start=True, stop=True)
                lsb = fp.tile([1, 1], F32, tag="lsb")
                nc.vector.tensor_scalar_mul(lsb[:], psl[:], 1.0 / p.ep)
                nc.sync.dma_start(lossloc[:], lsb[:])
                nc.gpsimd.collective_compute(
                    "AllReduce", mybir.AluOpType.add, replica_groups=rg,
                    ins=[lossloc.opt()], outs=[lossout.opt()])
                nc.sync.dma_start(out_loss[:], lossout[:])
    return nc


# -------------------------------------------------------------- driver

def _make_in_maps(p, W1, b1, W2, b2, score_w, score_b):
    iota = np.tile(np.arange(128, dtype=np.float32), (128, 1))
    ident = np.eye(128, dtype=np.float32)
    in_maps = []
    for c in range(NC):
        in_maps.append(dict(
            xT=p.xT[c],
            w1=np.asarray(W1, np.float32).astype(ml_dtypes.bfloat16),
            w2=np.asarray(W2, np.float32).astype(ml_dtypes.bfloat16),
            b1r=np.ascontiguousarray(
                np.tile(np.asarray(b1, np.float32).reshape(1, H), (128, 1))),
            b2r=np.ascontiguousarray(
                np.tile(np.asarray(b2, np.float32).reshape(1, C), (128, 1))),
            swr=np.ascontiguousarray(
                np.tile(np.asarray(score_w, np.float32).reshape(1, C), (128, 1))),
            sbr=np.full((128, 1), np.float32(np.asarray(score_b).reshape(-1)[0])),
            iota=iota, ident=ident,
            onesc=np.ones((128, 1), np.float32),
            degp=p.degarr[c],
            gidx=p.gidx_in[c], gtgt=p.gtgt_in[c],
            sidxs=p.sidx_s_in[c], sidxd=p.sidx_d_in[c],
        ))
    return in_maps


def _assemble(p, results):
    EP = p.ep
    out = np.zeros(EP, np.float32)
    for c in range(NC):
        o = np.asarray(results[c]["out_s"]).reshape(-1)
        m = p.slot2edge[c] >= 0
        out[p.slot2edge[c][m]] = o[m]
    loss = np.float32(results[0]["out_loss"][0, 0])
    return out, loss


def kernel(x, edge_index, pos_edge_index, neg_edge_index,
           W1, b1, W2, b2, score_w, score_b):
    global LAST_RESULTS
    _install_ntff_hook()
    x = np.asarray(x, np.float32)
    edge_index = np.asarray(edge_index)

    p = _build_plan(x, edge_index, np.asarray(pos_edge_index),
                    np.asarray(neg_edge_index))
    nc = _build_nc(p)
    nc.finalize()
    fix_dma_waits(nc)
    in_maps = _make_in_maps(p, W1, b1, W2, b2, score_w, score_b)
    res = run_bass_kernel_spmd(
        nc, in_maps, core_ids=list(range(NC)),
        trace=bool(os.environ.get("BASS_TRACE")))
    LAST_RESULTS = res
    return _assemble(p, [r for r in res.results])
